# revision 14
# baseline (speedup 1.0000x reference)
"""Trainium2 Bass kernel for nn_Node_GCN: out[n] = f(x[n]) + edge[n]^T @ g(cat(x,x)[n]).

Sharding: data-parallel over the batch dim N=8, one batch per NeuronCore.
Per core the dominant cost is streaming edge[n] from HBM once; activations
and edge are carried in fp16 (fp32 PSUM accumulation, fp32 biases), which
halves HBM traffic and keeps the PE at 1 cycle/row even when the HAM clock
gate is cold, so the kernel stays DMA-paced. All matmuls are laid out
feature-major so the contraction dim sits on SBUF partitions and edge
streams through the PE array as the moving operand.

The device computes outT[n] = [h, j]; the host transposes to [j, h] while
unsharding (gather-side layout fix, same as the x transpose on the way in).
"""

import numpy as np

D_IN = 64
D_HID = 128
M = 2048          # nodes per batch
N_BATCH = 8
NCORES = 8

# fp16 weights blob [128, WB_W]; rows 64:128 duplicate rows 0:64 for the
# K=64 matmuls whose rhs lives on partitions 64:128 (xT packed [128, 1024]).
_W_FW1 = 0          # f_w1 [64, 64]
_W_FW2 = 64         # f_w2 [64, 128]
_W_WG1 = 192        # wg1  [64, 128]  (= g_w1[:64] + g_w1[64:])
_W_GW2 = 320        # g_w2 [128, 128]
WB_W = 448

# fp32 bias blob [128, BB_W]
_B_GB2 = 0          # g_b2 broadcast rows, tiled 4x along free dim [128, 512]
_B_F1 = 512         # f_b1 [64, 1]
_B_G1 = 513         # g_b1 [128, 1]
_B_F2 = 514         # f_b2 [128, 1]
BB_W = 515

_NC_CACHE = {}


def _build():
    import concourse.bacc as bacc
    import concourse.mybir as mybir
    from concourse.tile import TileContext
    from concourse.bass import ts

    f32 = mybir.dt.float32
    f16 = mybir.dt.float16
    AF = mybir.ActivationFunctionType

    nc = bacc.Bacc()
    xT_d = nc.declare_dram_parameter("xT", [128, M // 2], f16, isOutput=False)
    edge_d = nc.declare_dram_parameter("edge", [M, M], f16, isOutput=False)
    wb_d = nc.declare_dram_parameter("wb", [128, WB_W], f16, isOutput=False)
    bb_d = nc.declare_dram_parameter("bb", [128, BB_W], f32, isOutput=False)
    outT_d = nc.declare_dram_parameter("outT", [D_HID, M], f32, isOutput=True)

    NT = M // 128    # 16 edge row-tiles
    NCH = M // 512   # 4 chunks of 512 for wide matmuls

    with TileContext(nc) as tc:
        with (
            tc.tile_pool(name="const", bufs=1) as cpool,
            tc.tile_pool(name="acts", bufs=1) as apool,
            tc.tile_pool(name="edgep", bufs=16) as epool,
            tc.tile_pool(name="pout", bufs=1, space="PSUM") as pout_pool,
            tc.tile_pool(name="pg", bufs=2, space="PSUM") as pg_pool,
            tc.tile_pool(name="pwork", bufs=2, space="PSUM") as pwork_pool,
        ):
            wb = cpool.tile([128, WB_W], f16, name="wb")
            bb = cpool.tile([128, BB_W], f32, name="bb")
            xT = cpool.tile([128, M // 2], f16, name="xT")
            # xT split in halves so the first h1g matmul waits on 0.13 MB only
            nc.sync.dma_start(out=xT[:, 0:512], in_=xT_d[:, 0:512])
            nc.sync.dma_start(out=wb, in_=wb_d[:])
            nc.sync.dma_start(out=bb, in_=bb_d[:])
            nc.sync.dma_start(out=xT[:, 512:1024], in_=xT_d[:, 512:1024])
            w_g2 = wb[0:128, _W_GW2:_W_GW2 + 128]
            gb2b4 = bb[0:128, _B_GB2:_B_GB2 + 512]
            b_f1 = bb[0:64, _B_F1:_B_F1 + 1]
            b_g1 = bb[0:128, _B_G1:_B_G1 + 1]
            b_f2 = bb[0:128, _B_F2:_B_F2 + 1]

            h1f = apool.tile([D_IN, M], f16, name="h1f")
            h1g = apool.tile([D_HID, M], f16, name="h1g")
            gx = apool.tile([128, M], f16, name="gx")  # tile i at [:, i*128:+128] is [t, h]
            outT = apool.tile([128, M], f32, name="outT")
            pout = pout_pool.tile([128, M], f32, name="pout")

            # warm the ACT function table during the preamble (hoists the lazy
            # ~1.3us ACT_TABLE_LOAD off the h1g critical path)
            warm = apool.tile([1, 1], f32, name="warm")
            nc.scalar.activation(warm, bb[0:1, 0:1], AF.Relu, bias=0.0)

            # k-th h1g chunk covers tokens 512k; (a, c2) = (k // 2, k % 2)
            def mm_h1g(k):
                a, c2 = divmod(k, 2)
                w_g1 = wb[64 * a:64 * a + 64, _W_WG1:_W_WG1 + 128]
                rhs = xT[64 * a:64 * a + 64, ts(c2, 512)]
                psg = pg_pool.tile([128, 512], f32, tag="g", name="psg")
                nc.tensor.matmul(psg, w_g1, rhs, start=True, stop=True)
                return psg, a * 1024 + c2 * 512

            def act_h1g(psg_tok):
                psg, tok = psg_tok
                nc.scalar.activation(h1g[:, tok:tok + 512], psg, AF.Relu, bias=b_g1)

            def h1f_chunk(k):
                a, c2 = divmod(k, 2)
                w_f1 = wb[64 * a:64 * a + 64, _W_FW1:_W_FW1 + 64]
                rhs = xT[64 * a:64 * a + 64, ts(c2, 512)]
                tok = a * 1024 + c2 * 512
                psf = pwork_pool.tile([64, 512], f32, tag="w", name="psf")
                nc.tensor.matmul(psf, w_f1, rhs, start=True, stop=True)
                nc.scalar.activation(h1f[:, tok:tok + 512], psf, AF.Relu, bias=b_f1)

            def gx_chunk(c):
                # gx tiles 4c..4c+3 (node-major [t, h]) batched: 4 matmuls into
                # one PSUM bank, one DVE bias-add
                psx = pwork_pool.tile([128, 512], f32, tag="w", name="psx")
                for k in range(4):
                    i = 4 * c + k
                    nc.tensor.matmul(
                        psx[:, ts(k, 128)], h1g[:, ts(i, 128)], w_g2,
                        start=True, stop=True,
                    )
                nc.vector.tensor_add(gx[:, ts(c, 512)], psx, gb2b4)

            def edge_iter(i):
                et = epool.tile([128, M], f16, tag="e", name="et")
                nc.sync.dma_start(out=et, in_=edge_d[ts(i, 128), :])
                for c in range(NCH):
                    nc.tensor.matmul(
                        pout[:, ts(c, 512)], gx[:, ts(i, 128)], et[:, ts(c, 512)],
                        start=(i == 0), stop=(i == NT - 1),
                    )

            # h1g token-chunk k unlocks gx tiles 4k..4k+3 (h1g free dim is the
            # token axis, gx tile i reads tokens 128i..128i+128). Interleave so
            # only chunk 0's chain gates the first edge matmuls; h1f and the
            # self-dynamics matmuls (joining pout's accumulation group
            # mid-stream) fill the early DMA-wait gaps.
            # token-chunk → xT half: k0,k2 need half 0; k1,k3 need half 1
            g0 = mm_h1g(0)
            g2 = mm_h1g(2)
            act_h1g(g0)
            gx_chunk(0)
            edge_iter(0); edge_iter(1)
            g1 = mm_h1g(1)
            act_h1g(g1)
            edge_iter(2); edge_iter(3)
            gx_chunk(1)
            g3 = mm_h1g(3)
            act_h1g(g2)
            h1f_chunk(0)
            edge_iter(4); edge_iter(5)
            act_h1g(g3)
            h1f_chunk(2)
            edge_iter(6)
            gx_chunk(2)
            h1f_chunk(1)
            edge_iter(7); edge_iter(8)
            h1f_chunk(3)
            edge_iter(9)
            w_f2 = wb[0:64, _W_FW2:_W_FW2 + 128]
            for c in range(NCH):
                nc.tensor.matmul(
                    pout[:, ts(c, 512)], w_f2, h1f[:, ts(c, 512)],
                    start=False, stop=False,
                )
            gx_chunk(3)
            for i in range(10, NT):
                edge_iter(i)

            # tail: per-chunk bias-add copy (alternating DVE/ACT), then one
            # full-bandwidth contiguous store (8KB/partition)
            for c in range(NCH):
                src = pout[:, ts(c, 512)]
                dst = outT[:, ts(c, 512)]
                if c % 2 == 0:
                    nc.vector.tensor_scalar_add(dst, src, b_f2)
                else:
                    nc.scalar.activation(dst, src, AF.Identity, bias=b_f2)
            nc.sync.dma_start(out=outT_d[:], in_=outT)
    nc.compile()
    return nc


def _get_nc():
    if "nc" not in _NC_CACHE:
        _NC_CACHE["nc"] = _build()
    return _NC_CACHE["nc"]


def _prep_in_maps(inputs):
    x = np.asarray(inputs["x"], dtype=np.float32)
    edge = np.asarray(inputs["edge"], dtype=np.float32)
    f_w1 = np.asarray(inputs["f_w1"], dtype=np.float32)
    f_b1 = np.asarray(inputs["f_b1"], dtype=np.float32)
    f_w2 = np.asarray(inputs["f_w2"], dtype=np.float32)
    f_b2 = np.asarray(inputs["f_b2"], dtype=np.float32)
    g_w1 = np.asarray(inputs["g_w1"], dtype=np.float32)
    g_b1 = np.asarray(inputs["g_b1"], dtype=np.float32)
    g_w2 = np.asarray(inputs["g_w2"], dtype=np.float32)
    g_b2 = np.asarray(inputs["g_b2"], dtype=np.float32)

    # cat(x, x) @ g_w1 == x @ (g_w1[:64] + g_w1[64:])
    wg1 = g_w1[:D_IN] + g_w1[D_IN:]

    wb = np.zeros((128, WB_W), dtype=np.float16)
    for r in (slice(0, 64), slice(64, 128)):  # duplicate for partition-64 rhs
        wb[r, _W_FW1:_W_FW1 + 64] = f_w1.astype(np.float16)
        wb[r, _W_FW2:_W_FW2 + 128] = f_w2.astype(np.float16)
        wb[r, _W_WG1:_W_WG1 + 128] = wg1.astype(np.float16)
    wb[0:128, _W_GW2:_W_GW2 + 128] = g_w2.astype(np.float16)

    bb = np.zeros((128, BB_W), dtype=np.float32)
    bb[0:128, _B_GB2:_B_GB2 + 512] = np.tile(g_b2[None, :], (128, 4))
    bb[0:64, _B_F1] = f_b1
    bb[0:128, _B_G1] = g_b1
    bb[0:128, _B_F2] = f_b2

    # x[n].T packed [128, 1024]: xT2[64a + k, t] = x[n, 1024a + t, k]
    xT = np.transpose(x, (0, 2, 1)).astype(np.float16)       # [8, 64, 2048]
    xT2 = np.concatenate([xT[:, :, :1024], xT[:, :, 1024:]], axis=1)  # [8, 128, 1024]
    xT2 = np.ascontiguousarray(xT2)
    edge16 = edge.astype(np.float16)
    in_maps = [
        {
            "xT": xT2[n],
            "edge": np.ascontiguousarray(edge16[n]),
            "wb": wb,
            "bb": bb,
        }
        for n in range(N_BATCH)
    ]
    return in_maps


def run(inputs, trace=False, **kw):
    """Run on 8 cores; returns (out [8, 2048, 128] fp32, BassKernelResults)."""
    from concourse.bass_utils import run_bass_kernel_spmd

    nc = _get_nc()
    in_maps = _prep_in_maps(inputs)
    res = run_bass_kernel_spmd(nc, in_maps, list(range(NCORES)), trace=trace, **kw)
    outT = np.stack([np.asarray(res.results[n]["outT"]) for n in range(N_BATCH)])
    out = np.ascontiguousarray(np.transpose(outT, (0, 2, 1)))  # [8, 2048, 128]
    return out.astype(np.float32, copy=False), res


def kernel(**inputs):
    out, _ = run(inputs, trace=False)
    return out


# revision 17
# speedup vs baseline: 1.1745x; 1.1745x over previous
"""Trainium2 Bass kernel for nn_Node_GCN: out[n] = f(x[n]) + edge[n]^T @ g(cat(x,x)[n]).

Sharding: data-parallel over the batch dim N=8, one batch per NeuronCore.
Per core the dominant cost is streaming edge[n] from HBM once; activations
and edge are carried in fp16 (fp32 PSUM accumulation, fp32 biases), which
halves HBM traffic and keeps the PE at 1 cycle/row even when the HAM clock
gate is cold, so the kernel stays DMA-paced. All matmuls are laid out
feature-major so the contraction dim sits on SBUF partitions and edge
streams through the PE array as the moving operand.

The device computes outT[n] = [h, j]; the host transposes to [j, h] while
unsharding (gather-side layout fix, same as the x transpose on the way in).
"""

import numpy as np

D_IN = 64
D_HID = 128
M = 2048          # nodes per batch
N_BATCH = 8
NCORES = 8

# fp16 weights blob [128, WB_W]; rows 64:128 duplicate rows 0:64 for the
# K=64 matmuls whose rhs lives on partitions 64:128 (xT packed [128, 1024]).
_W_FW1 = 0          # f_w1 [64, 64]
_W_FW2 = 64         # f_w2 [64, 128]
_W_WG1 = 192        # wg1  [64, 128]  (= g_w1[:64] + g_w1[64:])
_W_GW2 = 320        # g_w2 [128, 128]
WB_W = 448

# fp32 bias blob [128, BB_W]
_B_GB2 = 0          # g_b2 broadcast rows, tiled 4x along free dim [128, 512]
_B_F1 = 512         # f_b1 [64, 1]
_B_G1 = 513         # g_b1 [128, 1]
_B_F2 = 514         # f_b2 [128, 1]
BB_W = 515

_NC_CACHE = {}


def _build():
    import concourse.bacc as bacc
    import concourse.mybir as mybir
    from concourse.tile import TileContext
    from concourse.bass import ts

    f32 = mybir.dt.float32
    f16 = mybir.dt.float16
    AF = mybir.ActivationFunctionType

    nc = bacc.Bacc()
    xT_d = nc.declare_dram_parameter("xT", [128, M // 2], f16, isOutput=False)
    edge_d = nc.declare_dram_parameter("edge", [M, M], f16, isOutput=False)
    wb_d = nc.declare_dram_parameter("wb", [128, WB_W], f16, isOutput=False)
    bb_d = nc.declare_dram_parameter("bb", [128, BB_W], f32, isOutput=False)
    outT_d = nc.declare_dram_parameter("outT", [D_HID, M], f32, isOutput=True)

    NT = M // 128    # 16 edge row-tiles
    NCH = M // 512   # 4 chunks of 512 for wide matmuls

    with TileContext(nc) as tc:
        with (
            tc.tile_pool(name="const", bufs=1) as cpool,
            tc.tile_pool(name="acts", bufs=1) as apool,
            tc.tile_pool(name="edgep", bufs=16) as epool,
            tc.tile_pool(name="pout", bufs=1, space="PSUM") as pout_pool,
            tc.tile_pool(name="pg", bufs=2, space="PSUM") as pg_pool,
            tc.tile_pool(name="pwork", bufs=2, space="PSUM") as pwork_pool,
        ):
            wb = cpool.tile([128, WB_W], f16, name="wb")
            bb = cpool.tile([128, BB_W], f32, name="bb")
            xT = cpool.tile([128, M // 2], f16, name="xT")
            # xT split in halves so the first h1g matmul waits on 0.13 MB only
            nc.sync.dma_start(out=xT[:, 0:512], in_=xT_d[:, 0:512])
            nc.sync.dma_start(out=wb, in_=wb_d[:])
            nc.sync.dma_start(out=bb, in_=bb_d[:])
            nc.sync.dma_start(out=xT[:, 512:1024], in_=xT_d[:, 512:1024])
            w_g2 = wb[0:128, _W_GW2:_W_GW2 + 128]
            gb2b4 = bb[0:128, _B_GB2:_B_GB2 + 512]
            b_f1 = bb[0:64, _B_F1:_B_F1 + 1]
            b_g1 = bb[0:128, _B_G1:_B_G1 + 1]
            b_f2 = bb[0:128, _B_F2:_B_F2 + 1]

            h1f = apool.tile([D_IN, M], f16, name="h1f")
            h1g = apool.tile([D_HID, M], f16, name="h1g")
            gx = apool.tile([128, M], f16, name="gx")  # tile i at [:, i*128:+128] is [t, h]
            outT = apool.tile([128, M], f32, name="outT")
            pout = pout_pool.tile([128, M], f32, name="pout")

            # warm the ACT function table during the preamble (hoists the lazy
            # ~1.3us ACT_TABLE_LOAD off the h1g critical path)
            warm = apool.tile([1, 1], f32, name="warm")
            nc.scalar.activation(warm, bb[0:1, 0:1], AF.Relu, bias=0.0)

            # k-th h1g chunk covers tokens 512k; (a, c2) = (k // 2, k % 2)
            def mm_h1g(k):
                a, c2 = divmod(k, 2)
                w_g1 = wb[64 * a:64 * a + 64, _W_WG1:_W_WG1 + 128]
                rhs = xT[64 * a:64 * a + 64, ts(c2, 512)]
                psg = pg_pool.tile([128, 512], f32, tag="g", name="psg")
                nc.tensor.matmul(psg, w_g1, rhs, start=True, stop=True)
                return psg, a * 1024 + c2 * 512

            def act_h1g(psg_tok):
                psg, tok = psg_tok
                nc.scalar.activation(h1g[:, tok:tok + 512], psg, AF.Relu, bias=b_g1)

            def h1f_chunk(k):
                a, c2 = divmod(k, 2)
                w_f1 = wb[64 * a:64 * a + 64, _W_FW1:_W_FW1 + 64]
                rhs = xT[64 * a:64 * a + 64, ts(c2, 512)]
                tok = a * 1024 + c2 * 512
                psf = pwork_pool.tile([64, 512], f32, tag="w", name="psf")
                nc.tensor.matmul(psf, w_f1, rhs, start=True, stop=True)
                nc.scalar.activation(h1f[:, tok:tok + 512], psf, AF.Relu, bias=b_f1)

            def gx_chunk(c):
                # gx tiles 4c..4c+3 (node-major [t, h]) batched: 4 matmuls into
                # one PSUM bank, one DVE bias-add
                psx = pwork_pool.tile([128, 512], f32, tag="w", name="psx")
                for k in range(4):
                    i = 4 * c + k
                    nc.tensor.matmul(
                        psx[:, ts(k, 128)], h1g[:, ts(i, 128)], w_g2,
                        start=True, stop=True,
                    )
                nc.vector.tensor_add(gx[:, ts(c, 512)], psx, gb2b4)

            def edge_iter(i):
                et = epool.tile([128, M], f16, tag="e", name="et")
                nc.sync.dma_start(out=et, in_=edge_d[ts(i, 128), :])
                for c in range(NCH):
                    nc.tensor.matmul(
                        pout[:, ts(c, 512)], gx[:, ts(i, 128)], et[:, ts(c, 512)],
                        start=False, stop=(i == NT - 1),
                    )

            # h1g token-chunk k unlocks gx tiles 4k..4k+3 (h1g free dim is the
            # token axis, gx tile i reads tokens 128i..128i+128). Interleave so
            # only chunk 0's chain gates the first edge matmuls; h1f and the
            # self-dynamics matmuls (joining pout's accumulation group
            # mid-stream) fill the early DMA-wait gaps.
            # All MLP work first, then a gapless edge-matmul stream: while the
            # MLP chain runs (~7us), several edge chunks land in the 16
            # resident buffers, so the PE then runs the 64 edge matmuls
            # back-to-back — sustained-busy locks the HAM clock gate warm.
            # token-chunk → xT half: k0,k2 need half 0; k1,k3 need half 1
            g0 = mm_h1g(0)
            g2 = mm_h1g(2)
            act_h1g(g0)
            act_h1g(g2)
            g1 = mm_h1g(1)
            g3 = mm_h1g(3)
            act_h1g(g1)
            act_h1g(g3)
            h1f_chunk(0)
            h1f_chunk(2)
            h1f_chunk(1)
            h1f_chunk(3)
            for c in range(NCH):
                gx_chunk(c)
            # self-dynamics opens pout's accumulation group
            w_f2 = wb[0:64, _W_FW2:_W_FW2 + 128]
            for c in range(NCH):
                nc.tensor.matmul(
                    pout[:, ts(c, 512)], w_f2, h1f[:, ts(c, 512)],
                    start=True, stop=False,
                )
            for i in range(NT):
                edge_iter(i)

            # tail: per-chunk bias-add copy (alternating ACT/DVE) + store
            for c in range(NCH):
                src = pout[:, ts(c, 512)]
                dst = outT[:, ts(c, 512)]
                if c % 2 == 0:
                    nc.scalar.activation(dst, src, AF.Identity, bias=b_f2)
                else:
                    nc.vector.tensor_scalar_add(dst, src, b_f2)
                nc.sync.dma_start(out=outT_d[:, ts(c, 512)], in_=dst)
    nc.compile()
    return nc


def _get_nc():
    if "nc" not in _NC_CACHE:
        _NC_CACHE["nc"] = _build()
    return _NC_CACHE["nc"]


def _prep_in_maps(inputs):
    x = np.asarray(inputs["x"], dtype=np.float32)
    edge = np.asarray(inputs["edge"], dtype=np.float32)
    f_w1 = np.asarray(inputs["f_w1"], dtype=np.float32)
    f_b1 = np.asarray(inputs["f_b1"], dtype=np.float32)
    f_w2 = np.asarray(inputs["f_w2"], dtype=np.float32)
    f_b2 = np.asarray(inputs["f_b2"], dtype=np.float32)
    g_w1 = np.asarray(inputs["g_w1"], dtype=np.float32)
    g_b1 = np.asarray(inputs["g_b1"], dtype=np.float32)
    g_w2 = np.asarray(inputs["g_w2"], dtype=np.float32)
    g_b2 = np.asarray(inputs["g_b2"], dtype=np.float32)

    # cat(x, x) @ g_w1 == x @ (g_w1[:64] + g_w1[64:])
    wg1 = g_w1[:D_IN] + g_w1[D_IN:]

    wb = np.zeros((128, WB_W), dtype=np.float16)
    for r in (slice(0, 64), slice(64, 128)):  # duplicate for partition-64 rhs
        wb[r, _W_FW1:_W_FW1 + 64] = f_w1.astype(np.float16)
        wb[r, _W_FW2:_W_FW2 + 128] = f_w2.astype(np.float16)
        wb[r, _W_WG1:_W_WG1 + 128] = wg1.astype(np.float16)
    wb[0:128, _W_GW2:_W_GW2 + 128] = g_w2.astype(np.float16)

    bb = np.zeros((128, BB_W), dtype=np.float32)
    bb[0:128, _B_GB2:_B_GB2 + 512] = np.tile(g_b2[None, :], (128, 4))
    bb[0:64, _B_F1] = f_b1
    bb[0:128, _B_G1] = g_b1
    bb[0:128, _B_F2] = f_b2

    # x[n].T packed [128, 1024]: xT2[64a + k, t] = x[n, 1024a + t, k]
    xT = np.transpose(x, (0, 2, 1)).astype(np.float16)       # [8, 64, 2048]
    xT2 = np.concatenate([xT[:, :, :1024], xT[:, :, 1024:]], axis=1)  # [8, 128, 1024]
    xT2 = np.ascontiguousarray(xT2)
    edge16 = edge.astype(np.float16)
    in_maps = [
        {
            "xT": xT2[n],
            "edge": np.ascontiguousarray(edge16[n]),
            "wb": wb,
            "bb": bb,
        }
        for n in range(N_BATCH)
    ]
    return in_maps


def run(inputs, trace=False, **kw):
    """Run on 8 cores; returns (out [8, 2048, 128] fp32, BassKernelResults)."""
    from concourse.bass_utils import run_bass_kernel_spmd

    nc = _get_nc()
    in_maps = _prep_in_maps(inputs)
    res = run_bass_kernel_spmd(nc, in_maps, list(range(NCORES)), trace=trace, **kw)
    outT = np.stack([np.asarray(res.results[n]["outT"]) for n in range(N_BATCH)])
    out = np.ascontiguousarray(np.transpose(outT, (0, 2, 1)))  # [8, 2048, 128]
    return out.astype(np.float32, copy=False), res


def kernel(**inputs):
    out, _ = run(inputs, trace=False)
    return out
